# revision 1
# baseline (speedup 1.0000x reference)
"""Trainium2 Bass kernel for nn_Critic (dense MLP critic, 4 layers + LayerNorms).

Strategy (pure data parallel over 8 NeuronCores):
  - batch B=32768 sharded 8x -> 4096 rows/core; weights replicated.
  - all activations kept feature-major ([features on partitions, batch on
    free dim]) so the contraction dim of every matmul is the partition dim.
  - LayerNorm folded into the matmuls: for y = LN(z; g, beta) @ W.T + b,
      y[o,b] = invs[b]*( (W*g)z[o,b] - mu[b]*rowsum(W*g)[o] ) + (W@beta+b)[o]
    realized as an augmented matmul: activations get two extra K-rows
    (-mu[b], sigma[b]) and the weight matrix two extra rows
    (rowsum(W*g)[o], (W@beta+b)[o]); then h = tanh(invs (.) psum).
  - L1 stats (mean/var over 2080 features) via bn_stats on a second,
    batch-major copy of z; transposed to rows via a tiny PE transpose.
  - L2/L3 stats via (+-1/512)-ones-vector matmuls on PE (sum and sum-of-
    squares of h), with h^2 from ACT Square.
  - fp16 data everywhere (weights, activations), f32 PSUM/statistics.
"""

import os
import sys
import numpy as np

for _p in ("/opt/trn_rl_repo",):
    if os.path.isdir(_p) and _p not in sys.path:
        sys.path.append(_p)

from contextlib import ExitStack

import concourse.bass as bass  # noqa: E402
import concourse.tile as tile  # noqa: E402
from concourse import bacc, mybir  # noqa: E402
from concourse.bass_utils import run_bass_kernel_spmd  # noqa: E402

NCORES = 8
B = 32768
BC = B // NCORES  # rows per core
INPUT_DIM = 2048
HALF = INPUT_DIM // 2
N_ACTIONS = 32
D = INPUT_DIM + N_ACTIONS  # 2080
H = 512
NT = 512  # batch columns per tile
EPS = 1e-5
X_NORM = 50.0
V_NORM = 10.0

F16 = mybir.dt.float16
F32 = mybir.dt.float32
AF = mybir.ActivationFunctionType

K1 = 17  # ceil(D/128); last chunk has 32 data rows + 2 aug rows
K1_LAST = D - 16 * 128  # 32


def build_nc(bout: float, bc: int = BC):
    """Build + compile the per-core program. bc = rows per core."""
    ntiles = bc // NT
    assert ntiles * NT == bc

    nc = bacc.Bacc("TRN2", target_bir_lowering=False, debug=False,
                   num_devices=NCORES)

    zr_d = nc.dram_tensor("zr", [bc, D], F16, kind="ExternalInput").ap()
    zt_d = nc.dram_tensor("zt", [D, bc], F16, kind="ExternalInput").ap()
    w1_d = nc.dram_tensor("w1a", [D + 2, H], F16, kind="ExternalInput").ap()
    w2_d = nc.dram_tensor("w2a", [H + 2, H], F16, kind="ExternalInput").ap()
    w3_d = nc.dram_tensor("w3a", [H + 2, H], F16, kind="ExternalInput").ap()
    wo_d = nc.dram_tensor("wout", [H, 1], F16, kind="ExternalInput").ap()
    id_d = nc.dram_tensor("ident", [128, 128], F32, kind="ExternalInput").ap()
    q_d = nc.dram_tensor("q", [1, bc], F32, kind="ExternalOutput").ap()

    with tile.TileContext(nc) as tc:
        _emit(tc, ntiles, bout, zr_d, zt_d, w1_d, w2_d, w3_d, wo_d, id_d, q_d)

    nc.compile()
    return nc


def _emit(tc, ntiles, bout, zr_d, zt_d, w1_d, w2_d, w3_d, wo_d, id_d, q_d):
    nc = tc.nc
    with ExitStack() as ctx:
        wp = ctx.enter_context(tc.tile_pool(name="wp", bufs=1))
        zt_p = ctx.enter_context(tc.tile_pool(name="ztp", bufs=2))
        zr_p = ctx.enter_context(tc.tile_pool(name="zrp", bufs=2))
        h_p = ctx.enter_context(tc.tile_pool(name="hp", bufs=2))
        u_p = ctx.enter_context(tc.tile_pool(name="up", bufs=3))
        sq_p = ctx.enter_context(tc.tile_pool(name="sqp", bufs=3))
        bc_p = ctx.enter_context(tc.tile_pool(name="bcp", bufs=2))
        st_p = ctx.enter_context(tc.tile_pool(name="stp", bufs=3))
        ps_y = ctx.enter_context(tc.tile_pool(name="psy", bufs=3, space="PSUM"))
        ps_s = ctx.enter_context(tc.tile_pool(name="pss", bufs=1, space="PSUM"))
        ps_t = ctx.enter_context(tc.tile_pool(name="pst", bufs=2, space="PSUM"))
        ps_q = ctx.enter_context(tc.tile_pool(name="psq", bufs=1, space="PSUM"))

        # ---- persistent constants / weights ----
        w1 = []
        for k in range(K1):
            rows = 128 if k < 16 else K1_LAST + 2
            t = wp.tile([rows, H], F16, tag=f"w1_{k}")
            nc.sync.dma_start(out=t[:, :], in_=w1_d[k * 128:k * 128 + rows, :])
            w1.append(t)
        w2 = []
        w3 = []
        for name, wd, lst in (("w2", w2_d, w2), ("w3", w3_d, w3)):
            for k in range(4):
                t = wp.tile([128, H], F16, tag=f"{name}_{k}")
                nc.sync.dma_start(out=t[:, :], in_=wd[k * 128:(k + 1) * 128, :])
                lst.append(t)
            # rows H (rs) and H+1 (c) as separate [1, H] tiles
            for j in range(2):
                t = wp.tile([1, H], F16, tag=f"{name}_aug{j}")
                nc.sync.dma_start(out=t[:, :], in_=wd[H + j:H + j + 1, :])
                lst.append(t)
        wo = wp.tile([128, 4], F16, tag="wo")
        for k in range(4):
            nc.sync.dma_start(out=wo[:, k:k + 1], in_=wo_d[k * 128:(k + 1) * 128, :])
        ident = wp.tile([128, 128], F32, tag="ident")
        nc.sync.dma_start(out=ident[:, :], in_=id_d[:, :])
        onesn = wp.tile([128, 1], F16, tag="onesn")
        nc.vector.memset(onesn[:, :], -1.0 / H)
        onesp = wp.tile([128, 1], F16, tag="onesp")
        nc.vector.memset(onesp[:, :], 1.0 / H)
        epsT = wp.tile([128, 1], F32, tag="epsT")
        nc.vector.memset(epsT[:, :], EPS)
        boutT = wp.tile([1, 1], F32, tag="boutT")
        nc.vector.memset(boutT[:, :], bout)
        qrow = wp.tile([1, ntiles * NT], F32, tag="qrow")

        def evac(py, bctile, htile):
            """h = tanh(invs (.) psum) : DVE multiply + ACT tanh."""
            u = u_p.tile([128, NT], F16, tag="u")
            nc.vector.tensor_mul(u[:, :], py[:, :], bctile[:, :])
            nc.scalar.activation(htile[:, :], u[:, :], AF.Tanh)

        def bcast(row_ap):
            t = bc_p.tile([128, NT], F32, tag="bc")
            nc.gpsimd.partition_broadcast(t[:, :], row_ap)
            return t

        for it in range(ntiles):
            bs = it * NT

            # ---- L1 stats: bn_stats over batch-major z rows ----
            zt16 = zt_p.tile([K1_LAST + 2, NT], F16, tag="zt16")
            invs1 = st_p.tile([1, NT], F32, tag="invs1")
            zrt = zr_p.tile([128, 4, D], F16, tag="zrall")
            nc.sync.dma_start(out=zrt[:, :, :],
                              in_=zr_d[bs:bs + NT, :].rearrange("(c p) d -> p c d", c=4))
            for bch in range(4):
                stats = st_p.tile([128, 5, 6], F32, tag=f"st{bch}")
                zrv = zrt[:, bch, :].rearrange("p (n s) -> p n s", n=5)
                for i in range(5):
                    nc.vector.bn_stats(out=stats[:, i, :], in_=zrv[:, i, :])
                mv = st_p.tile([128, 2], F32, tag=f"mv{bch}")
                nc.vector.bn_aggr(out=mv[:, :], in_=stats[:, :, :])
                # pt cols: [sigma, -mu]; transposed rows pair with w1a aug
                # rows (c1, rs1) in that order.
                pt = st_p.tile([128, 2], F32, tag=f"pt{bch}")
                nc.scalar.activation(pt[:, 0:1], mv[:, 1:2], AF.Sqrt, bias=epsT[:, :])
                nc.vector.tensor_scalar_mul(pt[:, 1:2], mv[:, 0:1], -1.0)
                ptr = ps_t.tile([2, 128], F32, tag="ptr")
                nc.tensor.transpose(out=ptr[:, :], in_=pt[:, :], identity=ident[:, :])
                sl = slice(bch * 128, (bch + 1) * 128)
                nc.vector.tensor_copy(out=zt16[K1_LAST:K1_LAST + 2, sl], in_=ptr[0:2, :])
                nc.vector.reciprocal(invs1[0:1, sl], ptr[0:1, :])

            # ---- zT loads: one strided DMA for the 16 full chunks ----
            ztmain = zt_p.tile([128, 16, NT], F16, tag="ztmain")
            nc.sync.dma_start(
                out=ztmain[:, :, :],
                in_=zt_d[0:2048, bs:bs + NT].rearrange("(k p) n -> p k n", k=16))
            nc.sync.dma_start(out=zt16[0:K1_LAST, :], in_=zt_d[2048:2048 + K1_LAST, bs:bs + NT])
            zts = [ztmain[:, k, :] for k in range(16)] + [zt16]

            # ---- L1 matmuls + evac ----
            bc1 = bcast(invs1[0:1, :])
            h1 = []
            for m in range(4):
                py = ps_y.tile([128, NT], F32, tag="py")
                msl = slice(m * 128, (m + 1) * 128)
                for k in range(K1):
                    rk = zts[k] if k < 16 else zts[k][:, :]
                    nc.tensor.matmul(py[:, :], lhsT=w1[k][:, msl], rhs=rk,
                                     start=(k == 0), stop=(k == K1 - 1))
                ht = h_p.tile([128, NT], F16, tag=f"h1_{m}")
                evac(py, bc1, ht)
                h1.append(ht)

            # ---- L2 / L3 ----
            hcur = h1
            for lname, wts in (("l2", w2), ("l3", w3)):
                # stats: s1 = -mean, s2 = +E[h^2]
                s1 = ps_s.tile([1, NT], F32, tag="s1")
                s2 = ps_s.tile([1, NT], F32, tag="s2")
                for k in range(4):
                    nc.tensor.matmul(s1[:, :], lhsT=onesn[:, :], rhs=hcur[k][:, :],
                                     start=(k == 0), stop=(k == 3))
                for k in range(4):
                    sq = sq_p.tile([128, NT], F16, tag="sq")
                    nc.vector.tensor_mul(sq[:, :], hcur[k][:, :], hcur[k][:, :])
                    nc.tensor.matmul(s2[:, :], lhsT=onesp[:, :], rhs=sq[:, :],
                                     start=(k == 0), stop=(k == 3))
                musq = st_p.tile([1, NT], F32, tag="musq")
                nc.scalar.square(musq[:, :], s1[:, :])
                varr = st_p.tile([1, NT], F32, tag="var")
                nc.vector.tensor_sub(varr[:, :], s2[:, :], musq[:, :])
                negmu = h_p.tile([1, NT], F16, tag=f"negmu_{lname}")
                nc.vector.tensor_copy(out=negmu[:, :], in_=s1[:, :])
                sig32 = st_p.tile([1, NT], F32, tag="sig32")
                nc.scalar.activation(sig32[:, :], varr[:, :], AF.Sqrt, bias=epsT[0:1, :])
                sig16 = h_p.tile([1, NT], F16, tag=f"sig16_{lname}")
                nc.vector.tensor_copy(out=sig16[:, :], in_=sig32[:, :])
                invs = st_p.tile([1, NT], F32, tag="invs")
                nc.vector.reciprocal(invs[:, :], sig32[:, :])
                bct = bcast(invs[0:1, :])
                hnew = []
                for m in range(4):
                    py = ps_y.tile([128, NT], F32, tag="py")
                    msl = slice(m * 128, (m + 1) * 128)
                    for k in range(4):
                        nc.tensor.matmul(py[:, :], lhsT=wts[k][:, msl], rhs=hcur[k][:, :],
                                         start=(k == 0), stop=False)
                    nc.tensor.matmul(py[:, :], lhsT=wts[4][:, msl], rhs=negmu[:, :],
                                     start=False, stop=False)
                    nc.tensor.matmul(py[:, :], lhsT=wts[5][:, msl], rhs=sig16[:, :],
                                     start=False, stop=True)
                    ht = h_p.tile([128, NT], F16, tag=f"h_{lname}_{m}")
                    evac(py, bct, ht)
                    hnew.append(ht)
                hcur = hnew

            # ---- L4 ----
            pq = ps_q.tile([1, NT], F32, tag="pq")
            for k in range(4):
                nc.tensor.matmul(pq[:, :], lhsT=wo[:, k:k + 1], rhs=hcur[k][:, :],
                                 start=(k == 0), stop=(k == 3))
            nc.scalar.activation(qrow[0:1, bs:bs + NT], pq[:, :], AF.Tanh, bias=boutT[:, :])

        nc.sync.dma_start(out=q_d[:, :], in_=qrow[:, :])


# ---------------- host side ----------------

def host_prep(x, a, g1, beta1, g2, beta2, g3, beta3,
              w1, b1, w2, b2, w3, b3, w_out, b_out):
    """Shared (replicated) tensors + full z arrays; returns dict pieces."""
    f16 = np.float16
    z = np.empty((x.shape[0], D), dtype=f16)
    np.multiply(x[:, :HALF], np.float32(1.0 / X_NORM), out=z[:, :HALF], casting="unsafe")
    np.multiply(x[:, HALF:], np.float32(1.0 / V_NORM), out=z[:, HALF:INPUT_DIM], casting="unsafe")
    z[:, INPUT_DIM:] = a.astype(f16)

    def fold(w, g, beta, b, sigma_first):
        wg = (w.astype(np.float64) * g.astype(np.float64)[None, :])
        rs = wg.sum(axis=1)
        c = w.astype(np.float64) @ beta.astype(np.float64) + b.astype(np.float64)
        out = np.empty((w.shape[1] + 2, w.shape[0]), dtype=f16)
        out[:w.shape[1]] = wg.T.astype(f16)
        # L1 device aug rows arrive as (sigma, -mu) -> weight rows (c, rs);
        # L2/L3 use separate (negmu, sigma) rhs -> weight rows (rs, c).
        first, second = (c, rs) if sigma_first else (rs, c)
        out[w.shape[1]] = first.astype(f16)
        out[w.shape[1] + 1] = second.astype(f16)
        return out

    w1a = fold(w1, g1, beta1, b1, True)
    w2a = fold(w2, g2, beta2, b2, False)
    w3a = fold(w3, g3, beta3, b3, False)
    wout = w_out.T.astype(f16)  # [H, 1]
    bout = float(b_out[0])
    ident = np.eye(128, dtype=np.float32)
    return z, w1a, w2a, w3a, wout, bout, ident


_NC_CACHE = {}


def kernel(**inputs):
    inputs = {k: np.asarray(v) for k, v in inputs.items()}
    z, w1a, w2a, w3a, wout, bout, ident = host_prep(**inputs)

    key = (round(bout, 10), BC)
    if key not in _NC_CACHE:
        _NC_CACHE[key] = build_nc(bout, BC)
    nc = _NC_CACHE[key]

    in_maps = []
    for c in range(NCORES):
        zc = z[c * BC:(c + 1) * BC]
        in_maps.append({
            "zr": np.ascontiguousarray(zc),
            "zt": np.ascontiguousarray(zc.T),
            "w1a": w1a, "w2a": w2a, "w3a": w3a, "wout": wout, "ident": ident,
        })

    res = run_bass_kernel_spmd(nc, in_maps, list(range(NCORES)))
    q = np.concatenate([res.results[c]["q"].reshape(BC, 1) for c in range(NCORES)],
                       axis=0).astype(np.float32)
    return q



# revision 8
# speedup vs baseline: 180.5055x; 180.5055x over previous
"""Trainium2 Bass kernel for nn_Critic (dense MLP critic, 4 layers + LayerNorms).

Strategy (pure data parallel over 8 NeuronCores):
  - batch B=32768 sharded 8x -> 4096 rows/core; weights replicated.
  - activations feature-major ([features on partitions, batch on free dim])
    so every matmul contracts over the partition dim.
  - L1 LayerNorm folded into the matmul via two augmented K-rows:
      psum = (W*g)z + sigma*c1 - mu*rs1 ;  h1 = tanh(psum * (1/sigma))
    L1 stats via one 3-D bn_stats per 128-sample batch chunk on a
    batch-major copy of z; (sigma,-mu) transposed to rows via PE transpose.
    1/sigma via fp16 bit-trick seed + 2 fp32 Newton rsqrt iterations on
    per-partition [128,4] tiles (no ACT Sqrt -> no table swaps, no
    slow iterative row reciprocals).
  - L2/L3 stats via (+-1/512)-all-ones [128,128] matmuls, which yield
    mean/E[h^2] already broadcast across partitions (PSUM [128,512]) --
    no gpsimd broadcast needed.  1/sigma via f16 bit-trick Newton rsqrt
    on [128,512].  The 1/sigma scaling is applied to the matmul INPUT:
      hs = h * (1/sigma)
      psum = (W*g)hs + rs*(-mu/sigma) + c*1   (one K=2 aug matmul; the
        ones row lives in a persistent tile memset once)
      h = tanh(psum)        (ACT reads PSUM directly)
  - The ONLY ACT functions used are Tanh/Square/Copy -- all in the
    exp_and_others table set: one ACT table load for the whole kernel
    (the baseline paid 64 table swaps = ~82us).
  - fp16 data everywhere (weights, activations), f32 PSUM/statistics.
  - host ships z twice: batch-major (zr, for bn_stats) and feature-major
    pre-tiled (ztm/zt16) so each tile's load is one contiguous DMA with
    16KB-per-partition descriptors.
"""

import os
import sys
import numpy as np

for _p in ("/opt/trn_rl_repo",):
    if os.path.isdir(_p) and _p not in sys.path:
        sys.path.append(_p)

from contextlib import ExitStack

import concourse.bass as bass  # noqa: E402
import concourse.tile as tile  # noqa: E402
from concourse import bacc, mybir  # noqa: E402
from concourse.bass_utils import run_bass_kernel_spmd  # noqa: E402

NCORES = 8
B = 32768
BC = B // NCORES  # rows per core
INPUT_DIM = 2048
HALF = INPUT_DIM // 2
N_ACTIONS = 32
D = INPUT_DIM + N_ACTIONS  # 2080
H = 512
NT = 512  # batch columns per tile
EPS = 1e-5
X_NORM = 50.0
V_NORM = 10.0

F16 = mybir.dt.float16
F32 = mybir.dt.float32
I16 = mybir.dt.int16
AF = mybir.ActivationFunctionType
ALU = mybir.AluOpType

K1 = 17  # ceil(D/128); last chunk has 32 data rows + 2 aug rows
K1_LAST = D - 16 * 128  # 32

F16_RSQRT_MAGIC = 22971  # 0x59BB: fp16 fast-inverse-sqrt seed constant
F16_MIN_NORMAL = 6.2e-5  # clamp variance above fp16 subnormal range
NEWTON_ITERS = 2


def build_nc(bout: float, bc: int = BC):
    """Build + compile the per-core program. bc = rows per core."""
    ntiles = bc // NT
    assert ntiles * NT == bc

    nc = bacc.Bacc("TRN2", target_bir_lowering=False, debug=False,
                   num_devices=NCORES)

    zr_d = nc.dram_tensor("zr", [bc, D], F16, kind="ExternalInput").ap()
    ztm_d = nc.dram_tensor("ztm", [ntiles * 128, 16 * NT], F16,
                           kind="ExternalInput").ap()
    zt16_d = nc.dram_tensor("zt16", [ntiles * K1_LAST, NT], F16,
                            kind="ExternalInput").ap()
    w1_d = nc.dram_tensor("w1a", [D + 2, H], F16, kind="ExternalInput").ap()
    w2_d = nc.dram_tensor("w2a", [H + 2, H], F16, kind="ExternalInput").ap()
    w3_d = nc.dram_tensor("w3a", [H + 2, H], F16, kind="ExternalInput").ap()
    wo_d = nc.dram_tensor("wout", [128, 4], F16, kind="ExternalInput").ap()
    id_d = nc.dram_tensor("ident", [128, 128], F32, kind="ExternalInput").ap()
    q_d = nc.dram_tensor("q", [1, bc], F32, kind="ExternalOutput").ap()

    with tile.TileContext(nc) as tc:
        _emit(tc, ntiles, bout, zr_d, ztm_d, zt16_d, w1_d, w2_d, w3_d,
              wo_d, id_d, q_d)

    nc.compile()
    return nc


def _emit(tc, ntiles, bout, zr_d, ztm_d, zt16_d, w1_d, w2_d, w3_d,
          wo_d, id_d, q_d):
    nc = tc.nc
    with ExitStack() as ctx:
        wp = ctx.enter_context(tc.tile_pool(name="wp", bufs=1))
        zr_p = ctx.enter_context(tc.tile_pool(name="zrp", bufs=2))
        zt_p = ctx.enter_context(tc.tile_pool(name="ztp", bufs=2))
        z16_p = ctx.enter_context(tc.tile_pool(name="z16p", bufs=2))
        h_p = ctx.enter_context(tc.tile_pool(name="hp", bufs=2))
        hs_p = ctx.enter_context(tc.tile_pool(name="hsp", bufs=2))
        u_p = ctx.enter_context(tc.tile_pool(name="up", bufs=3))
        sq_p = ctx.enter_context(tc.tile_pool(name="sqp", bufs=3))
        bc_p = ctx.enter_context(tc.tile_pool(name="bcp", bufs=2))
        st_p = ctx.enter_context(tc.tile_pool(name="stp", bufs=3))
        nw_p = ctx.enter_context(tc.tile_pool(name="nwp", bufs=3))
        ps_y = ctx.enter_context(tc.tile_pool(name="psy", bufs=4, space="PSUM"))
        ps_s = ctx.enter_context(tc.tile_pool(name="pss", bufs=2, space="PSUM"))
        ps_t = ctx.enter_context(tc.tile_pool(name="pst", bufs=1, space="PSUM"))

        # ---- persistent constants / weights ----
        w1 = []
        for k in range(16):
            t = wp.tile([128, H], F16, tag=f"w1_{k}")
            nc.sync.dma_start(out=t[:, :], in_=w1_d[k * 128:(k + 1) * 128, :])
            w1.append(t)
        w1t = wp.tile([K1_LAST + 2, H], F16, tag="w1_tail")
        nc.sync.dma_start(out=w1t[:, :], in_=w1_d[2048:D + 2, :])

        w2, w3, aug = [], [], []
        for name, wd, lst in (("w2", w2_d, w2), ("w3", w3_d, w3)):
            for k in range(4):
                t = wp.tile([128, H], F16, tag=f"{name}_{k}")
                nc.sync.dma_start(out=t[:, :], in_=wd[k * 128:(k + 1) * 128, :])
                lst.append(t)
            a = wp.tile([2, H], F16, tag=f"{name}_aug")
            nc.sync.dma_start(out=a[:, :], in_=wd[H:H + 2, :])
            aug.append(a)
        wo = wp.tile([128, 4], F16, tag="wo")
        nc.sync.dma_start(out=wo[:, :], in_=wo_d[:, :])
        ident = wp.tile([128, 128], F32, tag="ident")
        nc.sync.dma_start(out=ident[:, :], in_=id_d[:, :])
        onesnJ = wp.tile([128, 128], F16, tag="onesnJ")
        nc.vector.memset(onesnJ[:, :], -1.0 / H)
        onespJ = wp.tile([128, 128], F16, tag="onespJ")
        nc.vector.memset(onespJ[:, :], 1.0 / H)
        boutT = wp.tile([1, 1], F32, tag="boutT")
        nc.vector.memset(boutT[:, :], bout)
        # persistent aug-rhs tiles; row1 stays 1.0 forever, row0 is
        # rewritten with -mu/sigma each (tile, layer).
        augr_t = []
        for li in range(2):
            pair = []
            for j in range(2):
                t = wp.tile([2, NT], F16, tag=f"augr_{li}_{j}")
                nc.vector.memset(t[:, :], 1.0)
                pair.append(t)
            augr_t.append(pair)
        qrow = wp.tile([1, ntiles * NT], F32, tag="qrow")

        def rsqrt_f16_seed(vc16, shape):
            """fp16 bit-trick rsqrt seed: y0i = MAGIC - (i >> 1)."""
            iv = vc16.bitcast(I16)
            t1 = nw_p.tile(shape, I16, tag="nw_t1", name="nw_t1")
            nc.vector.tensor_scalar(out=t1[:, :], in0=iv, scalar1=1,
                                    scalar2=-1, op0=ALU.logical_shift_right,
                                    op1=ALU.bitwise_xor)
            y0i = nw_p.tile(shape, I16, tag="nw_y0", name="nw_y0")
            nc.vector.tensor_scalar(out=y0i[:, :], in0=t1[:, :],
                                    scalar1=F16_RSQRT_MAGIC + 1, scalar2=None,
                                    op0=ALU.add)
            return y0i[:, :].bitcast(F16)

        def newton_iter(v_ap, y_ap, shape, dt, out_tile=None):
            """One Newton rsqrt iteration: y <- y*(1.5 - 0.5*v*y*y)."""
            a = nw_p.tile(shape, dt, tag="nw_a", name="nw_a")
            nc.vector.tensor_mul(a[:, :], y_ap, y_ap)
            b = nw_p.tile(shape, dt, tag="nw_b", name="nw_b")
            nc.vector.tensor_mul(b[:, :], v_ap, a[:, :])
            c = nw_p.tile(shape, dt, tag="nw_c", name="nw_c")
            nc.vector.tensor_scalar(out=c[:, :], in0=b[:, :], scalar1=-0.5,
                                    scalar2=1.5, op0=ALU.mult, op1=ALU.add)
            y1 = out_tile if out_tile is not None else nw_p.tile(
                shape, dt, tag="nw_y", name="nw_y")
            nc.vector.tensor_mul(y1[:, :], y_ap, c[:, :])
            return y1

        for it in range(ntiles):
            bs = it * NT

            # ---- L1 stats: bn_stats over batch-major z rows ----
            zrt = zr_p.tile([128, 4, D], F16, tag="zrall")
            nc.sync.dma_start(out=zrt[:, :, :],
                              in_=zr_d[bs:bs + NT, :].rearrange("(c p) d -> p c d", c=4))
            zt16 = z16_p.tile([K1_LAST + 2, NT], F16, tag="zt16")
            invrow = st_p.tile([1, NT], F32, tag="invrow")
            mvall = st_p.tile([128, 4, 2], F32, tag="mvall")
            for bch in range(4):
                stats = st_p.tile([128, 5, 6], F32, tag="st")
                zrv = zrt[:, bch, :].rearrange("p (n s) -> p n s", n=5)
                for i in range(5):
                    nc.vector.bn_stats(out=stats[:, i, :], in_=zrv[:, i, :])
                nc.vector.bn_aggr(out=mvall[:, bch, :], in_=stats[:, :, :])
            # per-partition scalar math on [128,4]: v=var+eps, 1/sigma via
            # Newton rsqrt, sigma = v * (1/sigma), nmu = -mu.
            v1 = st_p.tile([128, 4], F32, tag="v1")
            nc.vector.tensor_scalar(out=v1[:, :], in0=mvall[:, :, 1],
                                    scalar1=EPS, scalar2=None, op0=ALU.add)
            v16 = st_p.tile([128, 4], F16, tag="v16")
            nc.vector.tensor_scalar(out=v16[:, :], in0=v1[:, :],
                                    scalar1=F16_MIN_NORMAL, scalar2=None,
                                    op0=ALU.max)
            y = rsqrt_f16_seed(v16[:, :], [128, 4])  # AP (f16 view)
            for _ in range(NEWTON_ITERS):
                y = newton_iter(v1[:, :], y, [128, 4], F32)[:, :]
            invcol = y  # [128,4] f32 AP
            # pts[:, b, 0] = sigma_b ; pts[:, b, 1] = -mu_b
            pts = st_p.tile([128, 4, 2], F32, tag="pts")
            nc.vector.tensor_mul(pts[:, :, 0], v1[:, :], invcol[:, :])
            nc.vector.tensor_scalar(out=pts[:, :, 1], in0=mvall[:, :, 0],
                                    scalar1=-1.0, scalar2=None, op0=ALU.mult)
            for bch in range(4):
                sl = slice(bch * 128, (bch + 1) * 128)
                ptr = ps_t.tile([2, 128], F32, tag="ptr")
                nc.tensor.transpose(out=ptr[:, :], in_=pts[:, bch, :],
                                    identity=ident[:, :])
                nc.scalar.copy(out=zt16[K1_LAST:K1_LAST + 2, sl], in_=ptr[0:2, :])
                pti = ps_t.tile([1, 128], F32, tag="pti")
                nc.tensor.transpose(out=pti[:, :], in_=invcol[:, bch:bch + 1],
                                    identity=ident[:, :])
                nc.scalar.copy(out=invrow[0:1, sl], in_=pti[0:1, :])
            bc1 = bc_p.tile([128, NT], F32, tag="bc1")
            nc.gpsimd.partition_broadcast(bc1[:, :], invrow[0:1, :])

            # ---- zT loads: one contiguous DMA for the 16 full chunks ----
            ztm = zt_p.tile([128, 16, NT], F16, tag="ztm")
            nc.sync.dma_start(
                out=ztm[:, :, :],
                in_=ztm_d[it * 128:(it + 1) * 128, :].rearrange(
                    "p (k n) -> p k n", k=16))
            nc.sync.dma_start(
                out=zt16[0:K1_LAST, :],
                in_=zt16_d[it * K1_LAST:(it + 1) * K1_LAST, :])

            # ---- L1 matmuls + evac ----
            h1 = []
            for m in range(4):
                py = ps_y.tile([128, NT], F32, tag="py")
                msl = slice(m * 128, (m + 1) * 128)
                for k in range(16):
                    nc.tensor.matmul(py[:, :], lhsT=w1[k][:, msl], rhs=ztm[:, k, :],
                                     start=(k == 0), stop=False)
                nc.tensor.matmul(py[:, :], lhsT=w1t[:, msl], rhs=zt16[:, :],
                                 start=False, stop=True)
                u = u_p.tile([128, NT], F16, tag="u")
                nc.vector.tensor_mul(u[:, :], py[:, :], bc1[:, :])
                ht = h_p.tile([128, NT], F16, tag=f"h1_{m}")
                nc.scalar.activation(ht[:, :], u[:, :], AF.Tanh)
                h1.append(ht)

            # ---- L2 / L3 ----
            hcur = h1
            for li, (wts, at) in enumerate(((w2, aug[0]), (w3, aug[1]))):
                # stats, broadcast across partitions by all-ones matmuls:
                # S1bc = -mean(h), S2bc = +E[h^2]
                s1 = ps_s.tile([128, NT], F32, tag="s")
                for k in range(4):
                    nc.tensor.matmul(s1[:, :], lhsT=onesnJ[:, :], rhs=hcur[k][:, :],
                                     start=(k == 0), stop=(k == 3))
                s2 = ps_s.tile([128, NT], F32, tag="s")
                for k in range(4):
                    sq = sq_p.tile([128, NT], F16, tag="sq")
                    nc.scalar.activation(sq[:, :], hcur[k][:, :], AF.Square)
                    nc.tensor.matmul(s2[:, :], lhsT=onespJ[:, :], rhs=sq[:, :],
                                     start=(k == 0), stop=(k == 3))
                musq = st_p.tile([128, NT], F32, tag="musq")
                nc.scalar.activation(musq[:, :], s1[:, :], AF.Square)
                # vc16 = clamp((s2 + eps) - mu^2) in f16
                vpe = st_p.tile([128, NT], F16, tag="vpe")
                nc.vector.scalar_tensor_tensor(
                    out=vpe[:, :], in0=s2[:, :], scalar=EPS, in1=musq[:, :],
                    op0=ALU.add, op1=ALU.subtract)
                vc16 = st_p.tile([128, NT], F16, tag="vc16")
                nc.vector.tensor_scalar(out=vc16[:, :], in0=vpe[:, :],
                                        scalar1=F16_MIN_NORMAL, scalar2=None,
                                        op0=ALU.max)
                yb = rsqrt_f16_seed(vc16[:, :], [128, NT])  # AP
                for itn in range(NEWTON_ITERS):
                    last = (itn == NEWTON_ITERS - 1)
                    ot = (bc_p.tile([128, NT], F16, tag="invsbc", name="invsbc")
                          if last else None)
                    yb = newton_iter(vc16[:, :], yb, [128, NT], F16,
                                     out_tile=ot)[:, :]
                invsbc = yb  # [128,NT] f16 AP
                # aug rhs row0 = (-mu) * (1/sigma); row1 stays 1.0
                augr = augr_t[li][it % 2]
                nc.vector.tensor_mul(augr[0:1, :], s1[0:1, :], invsbc[0:1, :])
                hsl = []
                for k in range(4):
                    hst = hs_p.tile([128, NT], F16, tag=f"hs_{k}")
                    nc.vector.tensor_mul(hst[:, :], hcur[k][:, :], invsbc[:, :])
                    hsl.append(hst)
                hnew = []
                for m in range(4):
                    py = ps_y.tile([128, NT], F32, tag="py")
                    msl = slice(m * 128, (m + 1) * 128)
                    nc.tensor.matmul(py[:, :], lhsT=at[:, msl], rhs=augr[:, :],
                                     start=True, stop=False)
                    for k in range(4):
                        nc.tensor.matmul(py[:, :], lhsT=wts[k][:, msl],
                                         rhs=hsl[k][:, :],
                                         start=False, stop=(k == 3))
                    ht = h_p.tile([128, NT], F16, tag=f"h{li + 2}_{m}")
                    nc.scalar.activation(ht[:, :], py[:, :], AF.Tanh)
                    hnew.append(ht)
                hcur = hnew

            # ---- L4 ----
            pq = ps_s.tile([1, NT], F32, tag="s")
            for k in range(4):
                nc.tensor.matmul(pq[:, :], lhsT=wo[:, k:k + 1], rhs=hcur[k][:, :],
                                 start=(k == 0), stop=(k == 3))
            nc.scalar.activation(qrow[0:1, bs:bs + NT], pq[:, :], AF.Tanh,
                                 bias=boutT[:, :])

        nc.sync.dma_start(out=q_d[:, :], in_=qrow[:, :])


# ---------------- host side ----------------

def host_prep(x, a, g1, beta1, g2, beta2, g3, beta3,
              w1, b1, w2, b2, w3, b3, w_out, b_out):
    """Shared (replicated) tensors + full z arrays; returns dict pieces."""
    f16 = np.float16
    z = np.empty((x.shape[0], D), dtype=f16)
    np.multiply(x[:, :HALF], np.float32(1.0 / X_NORM), out=z[:, :HALF], casting="unsafe")
    np.multiply(x[:, HALF:], np.float32(1.0 / V_NORM), out=z[:, HALF:INPUT_DIM], casting="unsafe")
    z[:, INPUT_DIM:] = a.astype(f16)

    def fold(w, g, beta, b, sigma_first):
        wg = (w.astype(np.float64) * g.astype(np.float64)[None, :])
        rs = wg.sum(axis=1)
        c = w.astype(np.float64) @ beta.astype(np.float64) + b.astype(np.float64)
        out = np.empty((w.shape[1] + 2, w.shape[0]), dtype=f16)
        out[:w.shape[1]] = wg.T.astype(f16)
        # L1 device aug rows arrive as (sigma, -mu) -> weight rows (c, rs);
        # L2/L3 use rhs rows (nmu/sigma, ones) -> weight rows (rs, c).
        first, second = (c, rs) if sigma_first else (rs, c)
        out[w.shape[1]] = first.astype(f16)
        out[w.shape[1] + 1] = second.astype(f16)
        return out

    w1a = fold(w1, g1, beta1, b1, True)
    w2a = fold(w2, g2, beta2, b2, False)
    w3a = fold(w3, g3, beta3, b3, False)
    wout = np.ascontiguousarray(w_out.reshape(4, 128).T.astype(f16))  # [128, 4]
    bout = float(b_out[0])
    ident = np.eye(128, dtype=np.float32)
    return z, w1a, w2a, w3a, wout, bout, ident


def core_inputs(z, w1a, w2a, w3a, wout, ident, c):
    """Per-core input map (tiled feature-major layouts built here)."""
    zc = z[c * BC:(c + 1) * BC]
    ntiles = BC // NT
    # ztm[t, p, k, n] = zc[t*NT + n, k*128 + p]
    ztm = np.ascontiguousarray(
        zc[:, :2048].reshape(ntiles, NT, 16, 128).transpose(0, 3, 2, 1)
    ).reshape(ntiles * 128, 16 * NT)
    # zt16[t, r, n] = zc[t*NT + n, 2048 + r]
    zt16 = np.ascontiguousarray(
        zc[:, 2048:].reshape(ntiles, NT, K1_LAST).transpose(0, 2, 1)
    ).reshape(ntiles * K1_LAST, NT)
    return {
        "zr": np.ascontiguousarray(zc),
        "ztm": ztm,
        "zt16": zt16,
        "w1a": w1a, "w2a": w2a, "w3a": w3a, "wout": wout, "ident": ident,
    }


_NC_CACHE = {}


def kernel(**inputs):
    inputs = {k: np.asarray(v) for k, v in inputs.items()}
    z, w1a, w2a, w3a, wout, bout, ident = host_prep(**inputs)

    key = (round(bout, 10), BC)
    if key not in _NC_CACHE:
        _NC_CACHE[key] = build_nc(bout, BC)
    nc = _NC_CACHE[key]

    in_maps = [core_inputs(z, w1a, w2a, w3a, wout, ident, c)
               for c in range(NCORES)]

    res = run_bass_kernel_spmd(nc, in_maps, list(range(NCORES)))
    q = np.concatenate([res.results[c]["q"].reshape(BC, 1) for c in range(NCORES)],
                       axis=0).astype(np.float32)
    return q


# revision 10
# speedup vs baseline: 207.0445x; 1.1470x over previous
"""Trainium2 Bass kernel for nn_Critic (dense MLP critic, 4 layers + LayerNorms).

Strategy (pure data parallel over 8 NeuronCores):
  - batch B=32768 sharded 8x -> 4096 rows/core; weights replicated.
  - activations feature-major ([features on partitions, batch on free dim])
    so every matmul contracts over the partition dim.
  - L1 LayerNorm folded into the matmul via two augmented K-rows:
      psum = (W*g)z + sigma*c1 - mu*rs1 ;  h1 = tanh(psum * (1/sigma))
    L1 stats via one 3-D bn_stats per 128-sample batch chunk on a
    batch-major copy of z; (sigma,-mu) transposed to rows via PE transpose.
    1/sigma via fp16 bit-trick seed + 2 fp32 Newton rsqrt iterations on
    per-partition [128,4] tiles (no ACT Sqrt -> no table swaps, no
    slow iterative row reciprocals).
  - L2/L3 stats via (+-1/512)-all-ones [128,128] matmuls, which yield
    mean/E[h^2] already broadcast across partitions (PSUM [128,512]) --
    no gpsimd broadcast needed.  1/sigma via f16 bit-trick Newton rsqrt
    on [128,512].  The 1/sigma scaling is applied to the matmul INPUT:
      hs = h * (1/sigma)
      psum = (W*g)hs + rs*(-mu/sigma) + c*1   (one K=2 aug matmul; the
        ones row lives in a persistent tile memset once)
      h = tanh(psum)        (ACT reads PSUM directly)
  - The ONLY ACT functions used are Tanh/Square/Copy -- all in the
    exp_and_others table set: one ACT table load for the whole kernel
    (the baseline paid 64 table swaps = ~82us).
  - fp16 data everywhere (weights, activations), f32 PSUM/statistics.
  - host ships z twice: batch-major (zr, for bn_stats) and feature-major
    pre-tiled (ztm/zt16) so each tile's load is one contiguous DMA with
    16KB-per-partition descriptors.
"""

import os
import sys
import numpy as np

for _p in ("/opt/trn_rl_repo",):
    if os.path.isdir(_p) and _p not in sys.path:
        sys.path.append(_p)

from contextlib import ExitStack

import concourse.bass as bass  # noqa: E402
import concourse.tile as tile  # noqa: E402
from concourse import bacc, mybir  # noqa: E402
from concourse.bass_utils import run_bass_kernel_spmd  # noqa: E402

NCORES = 8
B = 32768
BC = B // NCORES  # rows per core
INPUT_DIM = 2048
HALF = INPUT_DIM // 2
N_ACTIONS = 32
D = INPUT_DIM + N_ACTIONS  # 2080
H = 512
NT = 512  # batch columns per tile
EPS = 1e-5
X_NORM = 50.0
V_NORM = 10.0

F16 = mybir.dt.float16
F32 = mybir.dt.float32
I16 = mybir.dt.int16
AF = mybir.ActivationFunctionType
ALU = mybir.AluOpType

K1 = 17  # ceil(D/128); last chunk has 32 data rows + 2 aug rows
K1_LAST = D - 16 * 128  # 32

F16_RSQRT_MAGIC = 22971  # 0x59BB: fp16 fast-inverse-sqrt seed constant
F16_MIN_NORMAL = 6.2e-5  # clamp variance above fp16 subnormal range
NEWTON_ITERS = 2


def build_nc(bout: float, bc: int = BC):
    """Build + compile the per-core program. bc = rows per core."""
    ntiles = bc // NT
    assert ntiles * NT == bc

    nc = bacc.Bacc("TRN2", target_bir_lowering=False, debug=False,
                   num_devices=NCORES)

    zr_d = nc.dram_tensor("zr", [bc, D], F16, kind="ExternalInput").ap()
    ztm_d = nc.dram_tensor("ztm", [ntiles * 128, 16 * NT], F16,
                           kind="ExternalInput").ap()
    zt16_d = nc.dram_tensor("zt16", [ntiles * K1_LAST, NT], F16,
                            kind="ExternalInput").ap()
    w1_d = nc.dram_tensor("w1a", [D + 2, H], F16, kind="ExternalInput").ap()
    w2_d = nc.dram_tensor("w2a", [H + 2, H], F16, kind="ExternalInput").ap()
    w3_d = nc.dram_tensor("w3a", [H + 2, H], F16, kind="ExternalInput").ap()
    wo_d = nc.dram_tensor("wout", [128, 4], F16, kind="ExternalInput").ap()
    id_d = nc.dram_tensor("ident", [128, 128], F32, kind="ExternalInput").ap()
    q_d = nc.dram_tensor("q", [1, bc], F32, kind="ExternalOutput").ap()

    with tile.TileContext(nc) as tc:
        _emit(tc, ntiles, bout, zr_d, ztm_d, zt16_d, w1_d, w2_d, w3_d,
              wo_d, id_d, q_d)

    nc.compile()
    return nc


def _emit(tc, ntiles, bout, zr_d, ztm_d, zt16_d, w1_d, w2_d, w3_d,
          wo_d, id_d, q_d):
    nc = tc.nc
    with ExitStack() as ctx:
        wp = ctx.enter_context(tc.tile_pool(name="wp", bufs=1))
        zr_p = ctx.enter_context(tc.tile_pool(name="zrp", bufs=2))
        zt_p = ctx.enter_context(tc.tile_pool(name="ztp", bufs=2))
        z16_p = ctx.enter_context(tc.tile_pool(name="z16p", bufs=2))
        h_p = ctx.enter_context(tc.tile_pool(name="hp", bufs=2))
        hs_p = ctx.enter_context(tc.tile_pool(name="hsp", bufs=2))
        u_p = ctx.enter_context(tc.tile_pool(name="up", bufs=3))
        sq_p = ctx.enter_context(tc.tile_pool(name="sqp", bufs=3))
        bc_p = ctx.enter_context(tc.tile_pool(name="bcp", bufs=2))
        st_p = ctx.enter_context(tc.tile_pool(name="stp", bufs=3))
        nw_p = ctx.enter_context(tc.tile_pool(name="nwp", bufs=3))
        ps_y = ctx.enter_context(tc.tile_pool(name="psy", bufs=5, space="PSUM"))
        ps_s = ctx.enter_context(tc.tile_pool(name="pss", bufs=2, space="PSUM"))
        ps_t = ctx.enter_context(tc.tile_pool(name="pst", bufs=1, space="PSUM"))

        # ---- persistent constants / weights ----
        w1 = []
        for k in range(16):
            t = wp.tile([128, H], F16, tag=f"w1_{k}")
            nc.sync.dma_start(out=t[:, :], in_=w1_d[k * 128:(k + 1) * 128, :])
            w1.append(t)
        w1t = wp.tile([K1_LAST + 2, H], F16, tag="w1_tail")
        nc.sync.dma_start(out=w1t[:, :], in_=w1_d[2048:D + 2, :])

        w2, w3, aug = [], [], []
        for name, wd, lst in (("w2", w2_d, w2), ("w3", w3_d, w3)):
            for k in range(4):
                t = wp.tile([128, H], F16, tag=f"{name}_{k}")
                nc.sync.dma_start(out=t[:, :], in_=wd[k * 128:(k + 1) * 128, :])
                lst.append(t)
            a = wp.tile([2, H], F16, tag=f"{name}_aug")
            nc.sync.dma_start(out=a[:, :], in_=wd[H:H + 2, :])
            aug.append(a)
        wo = wp.tile([128, 4], F16, tag="wo")
        nc.sync.dma_start(out=wo[:, :], in_=wo_d[:, :])
        ident = wp.tile([128, 128], F32, tag="ident")
        nc.sync.dma_start(out=ident[:, :], in_=id_d[:, :])
        onesnJ = wp.tile([128, 128], F16, tag="onesnJ")
        nc.vector.memset(onesnJ[:, :], -1.0 / H)
        onespJ = wp.tile([128, 128], F16, tag="onespJ")
        nc.vector.memset(onespJ[:, :], 1.0 / H)
        boutT = wp.tile([1, 1], F32, tag="boutT")
        nc.vector.memset(boutT[:, :], bout)
        # persistent aug-rhs tiles; row1 stays 1.0 forever, row0 is
        # rewritten with -mu/sigma each (tile, layer).
        augr_t = []
        for li in range(2):
            pair = []
            for j in range(2):
                t = wp.tile([2, NT], F16, tag=f"augr_{li}_{j}")
                nc.vector.memset(t[:, :], 1.0)
                pair.append(t)
            augr_t.append(pair)
        qrow = wp.tile([1, ntiles * NT], F32, tag="qrow")

        def rsqrt_f16_seed(vc16, shape):
            """fp16 bit-trick rsqrt seed: y0i = MAGIC - (i >> 1)."""
            iv = vc16.bitcast(I16)
            t1 = nw_p.tile(shape, I16, tag="nw_t1", name="nw_t1")
            nc.vector.tensor_scalar(out=t1[:, :], in0=iv, scalar1=1,
                                    scalar2=-1, op0=ALU.logical_shift_right,
                                    op1=ALU.bitwise_xor)
            y0i = nw_p.tile(shape, I16, tag="nw_y0", name="nw_y0")
            nc.vector.tensor_scalar(out=y0i[:, :], in0=t1[:, :],
                                    scalar1=F16_RSQRT_MAGIC + 1, scalar2=None,
                                    op0=ALU.add)
            return y0i[:, :].bitcast(F16)

        def newton_iter(v_ap, y_ap, shape, dt, out_tile=None):
            """One Newton rsqrt iteration: y <- y*(1.5 - 0.5*v*y*y)."""
            a = nw_p.tile(shape, dt, tag="nw_a", name="nw_a")
            nc.vector.tensor_mul(a[:, :], y_ap, y_ap)
            b = nw_p.tile(shape, dt, tag="nw_b", name="nw_b")
            nc.vector.tensor_mul(b[:, :], v_ap, a[:, :])
            c = nw_p.tile(shape, dt, tag="nw_c", name="nw_c")
            nc.vector.tensor_scalar(out=c[:, :], in0=b[:, :], scalar1=-0.5,
                                    scalar2=1.5, op0=ALU.mult, op1=ALU.add)
            y1 = out_tile if out_tile is not None else nw_p.tile(
                shape, dt, tag="nw_y", name="nw_y")
            nc.vector.tensor_mul(y1[:, :], y_ap, c[:, :])
            return y1

        # ================= software-pipelined tile processing ============
        # Per-tile context; emission is interleaved across tile t and t+1 so
        # the PE stream has independent L1(t+1) matmuls to chew on while the
        # DVE stats->Newton->scale chain of tile t runs (keeps HAM warm).

        def new_ct(it):
            return {"it": it, "h1": [None] * 4, "py": {}, "h": None}

        def emit_tile_dmas(ct):
            it = ct["it"]
            bs = it * NT
            ztm = zt_p.tile([128, 16, NT], F16, tag="ztm", name="ztm")
            nc.sync.dma_start(
                out=ztm[:, :, :],
                in_=ztm_d[it * 128:(it + 1) * 128, :].rearrange(
                    "p (k n) -> p k n", k=16))
            zt16 = z16_p.tile([K1_LAST + 2, NT], F16, tag="zt16", name="zt16")
            nc.sync.dma_start(
                out=zt16[0:K1_LAST, :],
                in_=zt16_d[it * K1_LAST:(it + 1) * K1_LAST, :])
            zrt = zr_p.tile([128, 4, D], F16, tag="zrall", name="zrall")
            nc.sync.dma_start(out=zrt[:, :, :],
                              in_=zr_d[bs:bs + NT, :].rearrange("(c p) d -> p c d", c=4))
            ct.update(ztm=ztm, zt16=zt16, zrt=zrt)

        def emit_stats_dve(ct):
            """bn_stats + per-partition Newton rsqrt; DVE-only (no PE)."""
            zrt = ct["zrt"]
            mvall = st_p.tile([128, 4, 2], F32, tag="mvall", name="mvall")
            for bch in range(4):
                stats = st_p.tile([128, 5, 6], F32, tag="st", name="st")
                zrv = zrt[:, bch, :].rearrange("p (n s) -> p n s", n=5)
                for i in range(5):
                    nc.vector.bn_stats(out=stats[:, i, :], in_=zrv[:, i, :])
                nc.vector.bn_aggr(out=mvall[:, bch, :], in_=stats[:, :, :])
            v1 = st_p.tile([128, 4], F32, tag="v1", name="v1")
            nc.vector.tensor_scalar(out=v1[:, :], in0=mvall[:, :, 1],
                                    scalar1=EPS, scalar2=None, op0=ALU.add)
            v16 = st_p.tile([128, 4], F16, tag="v16", name="v16")
            nc.vector.tensor_scalar(out=v16[:, :], in0=v1[:, :],
                                    scalar1=F16_MIN_NORMAL, scalar2=None,
                                    op0=ALU.max)
            y = rsqrt_f16_seed(v16[:, :], [128, 4])
            for _ in range(NEWTON_ITERS):
                y = newton_iter(v1[:, :], y, [128, 4], F32)[:, :]
            invcol = y  # [128,4] f32 AP
            pts = st_p.tile([128, 4, 2], F32, tag="pts", name="pts")
            nc.vector.tensor_mul(pts[:, :, 0], v1[:, :], invcol[:, :])
            nc.vector.tensor_scalar(out=pts[:, :, 1], in0=mvall[:, :, 0],
                                    scalar1=-1.0, scalar2=None, op0=ALU.mult)
            ct.update(pts=pts, invcol=invcol)

        def emit_stats_pe(ct):
            """Transposes + row assembly + bc1 (PE + ACT + gpsimd)."""
            zt16, pts, invcol = ct["zt16"], ct["pts"], ct["invcol"]
            invrow = st_p.tile([1, NT], F32, tag="invrow", name="invrow")
            for bch in range(4):
                sl = slice(bch * 128, (bch + 1) * 128)
                ptr = ps_t.tile([2, 128], F32, tag="ptr", name="ptr")
                nc.tensor.transpose(out=ptr[:, :], in_=pts[:, bch, :],
                                    identity=ident[:, :])
                nc.scalar.copy(out=zt16[K1_LAST:K1_LAST + 2, sl], in_=ptr[0:2, :])
                pti = ps_t.tile([1, 128], F32, tag="ptr", name="pti")
                nc.tensor.transpose(out=pti[:, :], in_=invcol[:, bch:bch + 1],
                                    identity=ident[:, :])
                nc.scalar.copy(out=invrow[0:1, sl], in_=pti[0:1, :])
            bc1 = bc_p.tile([128, NT], F32, tag="bc1", name="bc1")
            nc.gpsimd.partition_broadcast(bc1[:, :], invrow[0:1, :])
            ct["bc1"] = bc1

        def emit_l1_chain(ct, m):
            """k=0..15 accumulation (no zt16 dependency yet)."""
            py = ps_y.tile([128, NT], F32, tag="py", name="py")
            msl = slice(m * 128, (m + 1) * 128)
            for k in range(16):
                nc.tensor.matmul(py[:, :], lhsT=w1[k][:, msl],
                                 rhs=ct["ztm"][:, k, :],
                                 start=(k == 0), stop=False)
            ct["py"][m] = py

        def emit_l1_k16(ct, m):
            msl = slice(m * 128, (m + 1) * 128)
            nc.tensor.matmul(ct["py"][m][:, :], lhsT=w1t[:, msl],
                             rhs=ct["zt16"][:, :], start=False, stop=True)

        def emit_l1_evac(ct, m):
            u = u_p.tile([128, NT], F16, tag="u", name="u")
            nc.vector.tensor_mul(u[:, :], ct["py"][m][:, :], ct["bc1"][:, :])
            ht = h_p.tile([128, NT], F16, tag=f"h1_{m}", name=f"h1_{m}")
            nc.scalar.activation(ht[:, :], u[:, :], AF.Tanh)
            ct["h1"][m] = ht
            del ct["py"][m]

        def emit_l23_stats(ct, li):
            hcur = ct["h"]
            s1 = ps_s.tile([128, NT], F32, tag="s", name="s1")
            for k in range(4):
                nc.tensor.matmul(s1[:, :], lhsT=onesnJ[:, :], rhs=hcur[k][:, :],
                                 start=(k == 0), stop=(k == 3))
            s2 = ps_s.tile([128, NT], F32, tag="s", name="s2")
            for k in range(4):
                sq = sq_p.tile([128, NT], F16, tag="sq", name="sq")
                nc.scalar.activation(sq[:, :], hcur[k][:, :], AF.Square)
                nc.tensor.matmul(s2[:, :], lhsT=onespJ[:, :], rhs=sq[:, :],
                                 start=(k == 0), stop=(k == 3))
            ct["s1"], ct["s2"] = s1, s2

        def emit_l23_chain(ct, li):
            hcur, s1, s2 = ct["h"], ct["s1"], ct["s2"]
            musq = st_p.tile([128, NT], F32, tag="musq", name="musq")
            nc.scalar.activation(musq[:, :], s1[:, :], AF.Square)
            vc16 = st_p.tile([128, NT], F16, tag="vc16", name="vc16")
            nc.vector.scalar_tensor_tensor(
                out=vc16[:, :], in0=s2[:, :], scalar=EPS, in1=musq[:, :],
                op0=ALU.add, op1=ALU.subtract)
            yb = rsqrt_f16_seed(vc16[:, :], [128, NT])
            for itn in range(NEWTON_ITERS):
                last = (itn == NEWTON_ITERS - 1)
                ot = (bc_p.tile([128, NT], F16, tag="invsbc", name="invsbc")
                      if last else None)
                yb = newton_iter(vc16[:, :], yb, [128, NT], F16,
                                 out_tile=ot)[:, :]
            invsbc = yb
            augr = augr_t[li][ct["it"] % 2]
            nc.vector.tensor_mul(augr[0:1, :], s1[0:1, :], invsbc[0:1, :])
            hsl = []
            for k in range(4):
                hst = hs_p.tile([128, NT], F16, tag=f"hs_{k}", name=f"hs_{k}")
                nc.vector.tensor_mul(hst[:, :], hcur[k][:, :], invsbc[:, :])
                hsl.append(hst)
            ct["hsl"], ct["augr"] = hsl, augr

        def emit_l23_main(ct, li, wts, at):
            hsl, augr = ct["hsl"], ct["augr"]
            hnew = []
            for m in range(4):
                py = ps_y.tile([128, NT], F32, tag="py", name="py")
                msl = slice(m * 128, (m + 1) * 128)
                nc.tensor.matmul(py[:, :], lhsT=at[:, msl], rhs=augr[:, :],
                                 start=True, stop=False)
                for k in range(4):
                    nc.tensor.matmul(py[:, :], lhsT=wts[k][:, msl],
                                     rhs=hsl[k][:, :],
                                     start=False, stop=(k == 3))
                ht = h_p.tile([128, NT], F16, tag=f"h{li + 2}_{m}",
                              name=f"h{li + 2}_{m}")
                nc.scalar.activation(ht[:, :], py[:, :], AF.Tanh)
                hnew.append(ht)
            ct["h"] = hnew

        def emit_l4(ct):
            hcur = ct["h"]
            pq = ps_s.tile([1, NT], F32, tag="s", name="pq")
            for k in range(4):
                nc.tensor.matmul(pq[:, :], lhsT=wo[:, k:k + 1], rhs=hcur[k][:, :],
                                 start=(k == 0), stop=(k == 3))
            bs = ct["it"] * NT
            nc.scalar.activation(qrow[0:1, bs:bs + NT], pq[:, :], AF.Tanh,
                                 bias=boutT[:, :])

        # prologue: tile 0 data + stats
        ct = new_ct(0)
        emit_tile_dmas(ct)
        emit_stats_dve(ct)
        emit_stats_pe(ct)

        for it in range(ntiles):
            nct = None
            if it + 1 < ntiles:
                nct = new_ct(it + 1)
                emit_tile_dmas(nct)

            # ---- A: finish L1 of tile t ----
            if 1 in ct["py"]:  # m=1 prefilled by previous tile's filler
                emit_l1_k16(ct, 1)
                emit_l1_evac(ct, 1)
                todo = (2, 3)
            else:
                todo = (0, 1, 2, 3)
            for m in todo:
                emit_l1_chain(ct, m)
                emit_l1_k16(ct, m)
                emit_l1_evac(ct, m)
            ct["h"] = ct["h1"]

            # ---- B: L2 stats + chain ----
            emit_l23_stats(ct, 0)
            emit_l23_chain(ct, 0)
            # ---- C: PE gap filler: L1(t+1) m=0 partial chain ----
            if nct is not None:
                emit_l1_chain(nct, 0)
            # ---- D: L2 main ----
            emit_l23_main(ct, 0, w2, aug[0])
            # ---- E: next tile's DVE stats (after this tile's L2 chain) ----
            if nct is not None:
                emit_stats_dve(nct)
            # ---- F: L3 stats + chain ----
            emit_l23_stats(ct, 1)
            emit_l23_chain(ct, 1)
            # ---- G: fillers: L1(t+1) m=1 chain, stats-PE(t+1), m=0 finish --
            if nct is not None:
                emit_l1_chain(nct, 1)
                emit_stats_pe(nct)
                emit_l1_k16(nct, 0)
                emit_l1_evac(nct, 0)
            # ---- H: L3 main + L4 ----
            emit_l23_main(ct, 1, w3, aug[1])
            emit_l4(ct)
            ct = nct

        nc.sync.dma_start(out=q_d[:, :], in_=qrow[:, :])


# ---------------- host side ----------------

def host_prep(x, a, g1, beta1, g2, beta2, g3, beta3,
              w1, b1, w2, b2, w3, b3, w_out, b_out):
    """Shared (replicated) tensors + full z arrays; returns dict pieces."""
    f16 = np.float16
    z = np.empty((x.shape[0], D), dtype=f16)
    np.multiply(x[:, :HALF], np.float32(1.0 / X_NORM), out=z[:, :HALF], casting="unsafe")
    np.multiply(x[:, HALF:], np.float32(1.0 / V_NORM), out=z[:, HALF:INPUT_DIM], casting="unsafe")
    z[:, INPUT_DIM:] = a.astype(f16)

    def fold(w, g, beta, b, sigma_first):
        wg = (w.astype(np.float64) * g.astype(np.float64)[None, :])
        rs = wg.sum(axis=1)
        c = w.astype(np.float64) @ beta.astype(np.float64) + b.astype(np.float64)
        out = np.empty((w.shape[1] + 2, w.shape[0]), dtype=f16)
        out[:w.shape[1]] = wg.T.astype(f16)
        # L1 device aug rows arrive as (sigma, -mu) -> weight rows (c, rs);
        # L2/L3 use rhs rows (nmu/sigma, ones) -> weight rows (rs, c).
        first, second = (c, rs) if sigma_first else (rs, c)
        out[w.shape[1]] = first.astype(f16)
        out[w.shape[1] + 1] = second.astype(f16)
        return out

    w1a = fold(w1, g1, beta1, b1, True)
    w2a = fold(w2, g2, beta2, b2, False)
    w3a = fold(w3, g3, beta3, b3, False)
    wout = np.ascontiguousarray(w_out.reshape(4, 128).T.astype(f16))  # [128, 4]
    bout = float(b_out[0])
    ident = np.eye(128, dtype=np.float32)
    return z, w1a, w2a, w3a, wout, bout, ident


def core_inputs(z, w1a, w2a, w3a, wout, ident, c):
    """Per-core input map (tiled feature-major layouts built here)."""
    zc = z[c * BC:(c + 1) * BC]
    ntiles = BC // NT
    # ztm[t, p, k, n] = zc[t*NT + n, k*128 + p]
    ztm = np.ascontiguousarray(
        zc[:, :2048].reshape(ntiles, NT, 16, 128).transpose(0, 3, 2, 1)
    ).reshape(ntiles * 128, 16 * NT)
    # zt16[t, r, n] = zc[t*NT + n, 2048 + r]
    zt16 = np.ascontiguousarray(
        zc[:, 2048:].reshape(ntiles, NT, K1_LAST).transpose(0, 2, 1)
    ).reshape(ntiles * K1_LAST, NT)
    return {
        "zr": np.ascontiguousarray(zc),
        "ztm": ztm,
        "zt16": zt16,
        "w1a": w1a, "w2a": w2a, "w3a": w3a, "wout": wout, "ident": ident,
    }


_NC_CACHE = {}


def kernel(**inputs):
    inputs = {k: np.asarray(v) for k, v in inputs.items()}
    z, w1a, w2a, w3a, wout, bout, ident = host_prep(**inputs)

    key = (round(bout, 10), BC)
    if key not in _NC_CACHE:
        _NC_CACHE[key] = build_nc(bout, BC)
    nc = _NC_CACHE[key]

    in_maps = [core_inputs(z, w1a, w2a, w3a, wout, ident, c)
               for c in range(NCORES)]

    res = run_bass_kernel_spmd(nc, in_maps, list(range(NCORES)))
    q = np.concatenate([res.results[c]["q"].reshape(BC, 1) for c in range(NCORES)],
                       axis=0).astype(np.float32)
    return q
